# revision 3
# baseline (speedup 1.0000x reference)
"""LDAM hinge loss on 8 Trainium2 NeuronCores (Bass/Tile, data-parallel).

Reference math (per sample i, logits z0,z1, target t in {0,1}):
    d    = z1 - z0
    x    = (1-2t)*d + (t==0 ? D0 : D1)      # D0,D1 ~ 2-4e-6
    loss = sum_i softplus(x_i)              # softplus(x) = log(1+exp(x))

Device formulation (error < 4e-6 relative, dominated by fp32 anyway):
    softplus(-d+D1) = softplus(d-D1) - (d-D1), and since D0,D1 differ by
    ~6e-6 both branches evaluate softplus at w = d + (D0-D1)/2:
        loss ~= sum_i softplus(w_i) - sum_i t_i*(w_i - (D0+D1)/2)
    Per tile: one DVE scalar_tensor_tensor for w (strided reads of the
    interleaved logit pair), one DVE scalar_tensor_tensor for term B with
    fused per-partition accumulation (accum_out, int8 t operand), and one
    ACT Softplus with accum_out for term A.  3 ops per tile total.

Sharding strategy (host side): the loss is a plain data-parallel sum, so
the N samples are split contiguously across the 8 cores.  The int64
target values are all in {0,1} (the class labels of a binary LDAM loss),
so of the 8 little-endian bytes per target only the lowest is ever
nonzero; the shard layout therefore ships just that low byte per sample
(a pure numpy view+slice, no arithmetic) and the kernel streams 9
B/sample (8 B fp32 logit pair + 1 B label) instead of 16.  The kernel's
HBM traffic is 4.5 MiB/core, against a ~360-425 GB/s per-core roofline.

Device layout: partition p owns the 4096 consecutive samples
[p*4096, (p+1)*4096) of its core's shard.  The labels arrive in one
up-front [128, 4096] int8 DMA; the logits stream as [128, fk] f32
column-slices of the matching [128, 8192] view, on a shrinking tile
schedule (big tiles keep DMA at line rate, small final tiles cut the
post-last-byte compute tail).  Partial sums leave as two [128, nt] f32
grids; the host sums them in float64.
"""
import sys

sys.path.insert(0, "/opt/trn_rl_repo")

import numpy as np
import concourse.bacc as bacc
import concourse.mybir as mybir
from concourse.tile import TileContext
from concourse.bass_utils import run_bass_kernel_spmd

N = 4194304
N_CORES = 8
NP = N // N_CORES            # samples per core (524288)
P = 128
FD_TOTAL = (NP * 2) // P     # f32 elements per partition per core (8192)
FT = FD_TOTAL // 2           # label bytes per partition per core (4096)
TILE_SCHEDULE = [4096, 2048, 1024, 512, 512]
IO_BUFS = 2
MID_BUFS = 4

D0 = 0.5 / 30000.0 / 4.0     # delta for class 0  (C / (w0*n) / 4)
D1 = 0.5 / 70000.0 / 4.0     # delta for class 1

TRACE = False                # set by test harness to collect HW exec time
LAST = None                  # last BassKernelResults (for profiling)

_programs = {}


def _build(reps: int = 1, sched=None, io_bufs: int = IO_BUFS,
           mid_bufs: int = MID_BUFS, mode: str = "full",
           x_dma_engine: str = "sync", t_dma_engine: str = "scalar",
           rep_barrier: bool = False):
    """reps>1 repeats the whole per-core pipeline in the instruction stream
    (same data, same SBUF slots) — used only for timing-slope measurement.
    mode="dma" drops all compute (DMA floor ablation); x_dma_engine="alt"
    alternates x tiles between the two HWDGE rings; rep_barrier adds a
    strict all-engine scheduling barrier per rep."""
    f32 = mybir.dt.float32
    i8 = mybir.dt.int8
    Alu = mybir.AluOpType
    Act = mybir.ActivationFunctionType
    sched = list(sched) if sched is not None else list(TILE_SCHEDULE)
    assert sum(sched) == FD_TOTAL, sched
    nt = len(sched)

    nc = bacc.Bacc("TRN2", target_bir_lowering=False, debug=False)
    x_in = nc.declare_dram_parameter("x", [NP * 2], f32, isOutput=False)
    t_in = nc.declare_dram_parameter("t", [NP], i8, isOutput=False)
    accA_out = nc.declare_dram_parameter("accA", [P, nt], f32, isOutput=True)
    accB_out = nc.declare_dram_parameter("accB", [P, nt], f32, isOutput=True)
    x2 = x_in.rearrange("(p f) -> p f", f=FD_TOTAL)   # [128, 8192] f32
    t2 = t_in.rearrange("(p f) -> p f", f=FT)         # [128, 4096] i8

    with TileContext(nc) as tc:
        with (
            tc.tile_pool(name="io", bufs=io_bufs) as io,
            tc.tile_pool(name="tp", bufs=1) as tp,
            tc.tile_pool(name="mid", bufs=mid_bufs) as mid,
            tc.tile_pool(name="accp", bufs=1) as accp,
        ):
            accA = accp.tile([P, nt], f32)
            accB = accp.tile([P, nt], f32)
            if mode == "dma":
                nc.vector.memset(accA[:], 0.0)
                nc.vector.memset(accB[:], 0.0)
            t_eng = nc.scalar if t_dma_engine == "scalar" else nc.sync
            for _r in range(reps):
                if rep_barrier:
                    tc.strict_bb_all_engine_barrier()
                tt = tp.tile([P, FT], i8, tag="t")
                t_eng.dma_start(out=tt[:], in_=t2)
                c0 = 0
                for i, fk in enumerate(sched):
                    if x_dma_engine == "alt":
                        x_eng = nc.sync if i % 2 == 0 else nc.scalar
                    else:
                        x_eng = nc.sync if x_dma_engine == "sync" else nc.scalar
                    xt = io.tile([P, fk], f32, tag="x")
                    x_eng.dma_start(out=xt[:], in_=x2[:, c0 : c0 + fk])
                    if mode == "dma":
                        c0 += fk
                        continue
                    h = fk // 2
                    # w = (z1 + (D0-D1)/2) - z0
                    x0 = mid.tile([P, h], f32, tag="x0")
                    nc.vector.scalar_tensor_tensor(
                        out=x0[:], in0=xt[:, 1::2], scalar=float((D0 - D1) / 2.0),
                        in1=xt[:, 0::2], op0=Alu.add, op1=Alu.subtract,
                    )
                    # termB row-sums: sum_f t*(w - (D0+D1)/2)
                    jb = mid.tile([P, h], f32, tag="jb")
                    nc.vector.scalar_tensor_tensor(
                        out=jb[:], in0=x0[:], scalar=float(-(D0 + D1) / 2.0),
                        in1=tt[:, c0 // 2 : c0 // 2 + h], op0=Alu.add, op1=Alu.mult,
                        accum_out=accB[:, i : i + 1],
                    )
                    # termA row-sums: sum_f ln(exp(w) + 1)  (no Softplus
                    # table in this bass build; Exp and Ln share a set)
                    u = mid.tile([P, h], f32, tag="u")
                    nc.scalar.activation(out=u[:], in_=x0[:], func=Act.Exp)
                    ja = mid.tile([P, h], f32, tag="ja")
                    nc.scalar.activation(
                        out=ja[:], in_=u[:], func=Act.Ln, bias=1.0, scale=1.0,
                        accum_out=accA[:, i : i + 1],
                    )
                    c0 += fk
            # accB (last written by DVE) goes out on the SP ring while the
            # final Softplus still runs; accA follows on the ACT ring.
            nc.sync.dma_start(out=accB_out[:], in_=accB[:])
            nc.scalar.dma_start(out=accA_out[:], in_=accA[:])
    nc.compile()
    return nc


def _get_program():
    key = ("default", 1)
    if key not in _programs:
        _programs[key] = _build()
    return _programs[key]


def _shard_inputs(output, target):
    output = np.asarray(output)
    target = np.asarray(target)
    assert output.shape == (N, 2), output.shape
    xflat = np.ascontiguousarray(output, dtype=np.float32).reshape(-1)  # [2N]
    t = np.ascontiguousarray(target.reshape(-1))
    if t.dtype != np.int8:
        # labels are {0,1}: only the little-endian low byte is nonzero
        t = t.view(np.int8)[0 :: t.dtype.itemsize]
    in_maps = [
        {
            "x": xflat[c * NP * 2 : (c + 1) * NP * 2],
            "t": np.ascontiguousarray(t[c * NP : (c + 1) * NP]),
        }
        for c in range(N_CORES)
    ]
    return in_maps


def kernel(output, target):
    global LAST
    in_maps = _shard_inputs(output, target)
    nc = _get_program()
    try:
        LAST = run_bass_kernel_spmd(
            nc, in_maps, core_ids=list(range(N_CORES)), trace=TRACE
        )
    except ModuleNotFoundError:
        # axon NTFF hook unavailable in this environment: run untraced
        LAST = run_bass_kernel_spmd(
            nc, in_maps, core_ids=list(range(N_CORES)), trace=False
        )
    total = np.float64(0.0)
    for r in LAST.results:
        total += r["accA"].astype(np.float64).sum()
        total -= r["accB"].astype(np.float64).sum()
    return np.float32(total)
